# revision 3
# baseline (speedup 1.0000x reference)
"""Trainium2 Bass kernel for the DyadBlock problem.

Math (reference):
    xb   = x.reshape(DY, DI, B)
    incl = cumsum(xb, axis=0)             # inclusive prefix over dyads
    total= incl[-1]
    out[d] = w_lower[d] @ incl[d] + w_upper[d] @ (total - incl[d]) + bias

Rewrite (wd = w_lower - w_upper, T = total):
    out[d] = wd[d] @ incl[d] + w_upper[d] @ T + bias

Decomposition (per core; 64 row tiles of 4 dyads; 16 groups of 4 tiles):
    incl[4t+a] = local_prefix(tile t, dyad a) + C(t)       C(t) = prefix of
                                                           whole tiles < t
    out tile t = V[t].T @ x[t]                  (local cumsum folded into the
                                                 weight: one K=128 matmul
                                                 straight from x)
               + WF[t].T @ [C(t); T]            (carry AND total terms fused
                                                 into one K=64 partial-array
                                                 matmul; WF = [wd_t; wu_t])
               + bias                           (fused into the PSUM drain)
    C(t)/T come from a 16-step chain over per-group stacked tile totals
    (phase A, unchanged from the earlier 3-matmul kernel):
      stackP[j] = sum_a GW_a.T @ x[4j+a] + O4I.T @ stackP[j-1]

The [C;T] operands live in pooled "ct" tiles, two per group:
    ct[j][h] = [C(4j+2h); T; C(4j+2h+1); T]    (4 x 32 partitions)
built by partition-shifted DVE copies out of stackP once the chain resolves
(T = stackP[15][96:128]).  Tile t's fused matmul reads window
ct[j][ (t%4)//2 ][64*(t%2) : 64*(t%2)+64] with tile_position (64*(t%2), 0).

PSUM legality (found by bisection in the earlier session): a partial-array
(tile_position) matmul may be the LAST write into a PSUM bank after
full-array accumulation.  Each bank runs: V (start=True, full array) ->
fused (start=False, stop=True, K=64 partial, skip_group_check).

Schedule: x streams first on gpsimd (partition-major 2.1MB dma_starts);
phase A paces itself behind the stream; V and WF stream after x; ct copies
run on gpsimd (plus spillover) after the chain; output DMA (sync queue)
begins as soon as the first fused matmuls drain.  fp16 operands/IO
(PSUM fp32).
"""

import os

import numpy as np

import concourse.bacc as bacc
import concourse.mybir as mybir
import concourse.tile as tile
from concourse import bass_utils
from concourse.tile_rust import add_dep_helper

DY, DO, DI = 256, 32, 32
B = 8192
NCORES = 8
BC = B // NCORES   # batch columns per core
NT = DY // 4       # 64 row tiles (4 dyads each)
NG = NT // 4       # 16 groups (4 tiles each)

_cache = {}
last_results = None


def _cfg():
    mm = os.environ.get("DYAD_MM_DT", "fp16")
    mm_dt = {
        "f32": mybir.dt.float32,
        "f32r": mybir.dt.float32r,
        "bf16": mybir.dt.bfloat16,
        "fp16": mybir.dt.float16,
    }[mm]
    return (mm_dt,)


def build(mm_dt, bc=BC, n=512):
    f32 = mybir.dt.float32
    nchunk = bc // n
    nc = bacc.Bacc("TRN2", target_bir_lowering=False, debug=False,
                   num_devices=NCORES)
    x_d = nc.dram_tensor("x", [128, NT * bc], mm_dt,
                         kind="ExternalInput").ap()
    v_d = nc.dram_tensor("v", [128, NT * 128], mm_dt,
                         kind="ExternalInput").ap()
    wf_d = nc.dram_tensor("wf", [128, (NT // 2) * 128], mm_dt,
                          kind="ExternalInput").ap()
    gw_d = nc.dram_tensor("gw", [128, 4 * 128], mm_dt,
                          kind="ExternalInput").ap()
    o4i_d = nc.dram_tensor("o4i", [128, 128], mm_dt, kind="ExternalInput").ap()
    bias_d = nc.dram_tensor("biast", [128, NT], f32, kind="ExternalInput").ap()
    out_d = nc.dram_tensor("out", [NT * 128, bc], mm_dt,
                           kind="ExternalOutput").ap()

    with tile.TileContext(nc) as tc:
        with tc.tile_pool(name="wpool", bufs=1) as wp, \
             tc.tile_pool(name="xpool", bufs=NT // 8) as xp, \
             tc.tile_pool(name="sppool", bufs=nchunk * NG) as spp, \
             tc.tile_pool(name="ctpool", bufs=6) as ctp, \
             tc.tile_pool(name="opool", bufs=4) as op, \
             tc.tile_pool(name="psum", bufs=8, space="PSUM") as pp:
            gw = wp.tile([128, 4 * 128], mm_dt)
            o4i = wp.tile([128, 128], mm_dt)
            biast = wp.tile([128, NT], f32)
            v = wp.tile([128, NT * 128], mm_dt)
            wf = wp.tile([128, (NT // 2) * 128], mm_dt)

            nc.sync.dma_start(out=o4i[:], in_=o4i_d)
            nc.sync.dma_start(out=gw[:], in_=gw_d)
            nc.sync.dma_start(out=biast[:], in_=bias_d)

            # x ships partition-major ([128, NT*bc], host-transposed): eight
            # 2.1MB dma_starts with 16KB-contiguous per-partition lines.
            # Weights follow on the same queue (strictly after x).
            octs = []
            for o in range(NT // 8):
                oct_t = xp.tile([128, 8 * bc], mm_dt, tag="x",
                                name=f"xoct_{o}")
                nc.gpsimd.dma_start(
                    out=oct_t[:], in_=x_d[:, o * 8 * bc:(o + 1) * 8 * bc])
                octs.append(oct_t)

            def xsl(t, c0, c1):
                return octs[t // 8][:, (t % 8) * bc + c0:(t % 8) * bc + c1]

            # V / fused weights stream as quarters (into one resident tile
            # each) so phase-B group 4q never waits on quarters > q.
            QW = NT * 32          # V quarter width (16 tiles x 128 cols)
            QF = (NT // 2) * 32   # wf quarter width (16 tiles x 64 cols)
            for q in range(4):
                nc.gpsimd.dma_start(out=wf[:, q * QF:(q + 1) * QF],
                                    in_=wf_d[:, q * QF:(q + 1) * QF])
                nc.gpsimd.dma_start(out=v[:, q * QW:(q + 1) * QW],
                                    in_=v_d[:, q * QW:(q + 1) * QW])

            a_flip = 0

            def drain_a(dst, src):
                nonlocal a_flip
                if a_flip == 0:
                    nc.scalar.copy(out=dst, in_=src)
                else:
                    nc.vector.tensor_copy(out=dst, in_=src)
                a_flip ^= 1

            # ---- phase A: 16-step global chain, fed straight from x ----
            # stackP[j] = sum_a GW_a.T @ x[4j+a] + O4I.T @ stackP[j-1]
            # with GW_a = SUM4_a @ LT4 (stacked-total and in-group prefix
            # folded into one weight).
            stackP = [[None] * NG for _ in range(nchunk)]
            for j in range(NG):
                for c in range(nchunk):
                    sp_ps = pp.tile([128, n], f32, tag="ps",
                                    name=f"sp_{c}_{j}")
                    prev = None
                    for a in range(4):
                        m = nc.tensor.matmul(
                            sp_ps[:], gw[:, 128 * a:128 * (a + 1)],
                            xsl(4 * j + a, c * n, (c + 1) * n),
                            start=(a == 0), stop=(a == 3 and j == 0),
                            tile_position=(0, 0))
                        if prev is not None:
                            add_dep_helper(m.ins, prev.ins, sync=False,
                                           reason="psum chain accum order")
                        prev = m
                    if j > 0:
                        m2 = nc.tensor.matmul(sp_ps[:], o4i[:],
                                              stackP[c][j - 1][:],
                                              start=False, stop=True,
                                              tile_position=(0, 0))
                        add_dep_helper(m2.ins, prev.ins, sync=False,
                                       reason="psum chain order gw->m2")
                    sp_sb = spp.tile([128, n], mm_dt, tag="sp",
                                     name=f"spsb_{c}_{j}")
                    drain_a(sp_sb[:], sp_ps[:])
                    stackP[c][j] = sp_sb

            # ---- ct tiles: [C(4j+2h); T; C(4j+2h+1); T] per (j, h) ----
            # All copies depend on the chain end only through the T halves;
            # C halves read stackP[j]/stackP[j-1].  Copies ride gpsimd with
            # round-robin spillover so the out-drain engines stay free.
            ct_rot = 0

            def ct_copy(dst, src):
                nonlocal ct_rot
                eng = (nc.gpsimd, nc.gpsimd, nc.vector, nc.scalar)[ct_rot % 4]
                if eng is nc.scalar:
                    eng.copy(out=dst, in_=src)
                else:
                    eng.tensor_copy(out=dst, in_=src)
                ct_rot += 1

            cts = [[None, None] for _ in range(NG)]
            for j in range(NG):
                for h in range(2):
                    ctt = ctp.tile([128, bc], mm_dt, tag="ct",
                                   name=f"ct_{j}_{h}")
                    cts[j][h] = ctt
                    for p in range(2):
                        t = 4 * j + 2 * h + p
                        if t == 0:
                            nc.vector.memset(ctt[0:32, :], 0)
                        else:
                            srcg, s = (j - 1, 3) if t % 4 == 0 else \
                                      (j, t % 4 - 1)
                            for c in range(nchunk):
                                ct_copy(
                                    ctt[64 * p:64 * p + 32,
                                        c * n:(c + 1) * n],
                                    stackP[c][srcg][32 * s:32 * (s + 1), :])
                        for c in range(nchunk):
                            ct_copy(
                                ctt[64 * p + 32:64 * p + 64,
                                    c * n:(c + 1) * n],
                                stackP[c][NG - 1][96:128, :])

            # ---- phase B: V (full, start) -> fused [C;T] (K=64 partial,
            # stop) per PSUM bank, then bias-add drain + out DMA ----
            b_flip = 0
            for j in range(NG):
                outts = []
                for a in range(4):
                    outts.append(op.tile([128, bc], mm_dt, tag="out",
                                         name=f"out_{4 * j + a}"))
                for c in range(nchunk):
                    for a in range(4):
                        t = 4 * j + a
                        pout = pp.tile([128, n], f32, tag="ps",
                                       name=f"po_{c}_{t}")
                        mV = nc.tensor.matmul(
                            pout[:], v[:, 128 * t:128 * (t + 1)],
                            xsl(t, c * n, (c + 1) * n),
                            start=True, stop=False, tile_position=(0, 0))
                        row = 64 * (a % 2)
                        mF = nc.tensor.matmul(
                            pout[:],
                            wf[row:row + 64, 128 * (t // 2):
                               128 * (t // 2) + 128],
                            cts[j][a // 2][row:row + 64,
                                           c * n:(c + 1) * n],
                            start=False, stop=True,
                            tile_position=(row, 0),
                            skip_group_check=True)
                        add_dep_helper(mF.ins, mV.ins, sync=False,
                                       reason="psum order V->fused")
                        dst = outts[a][:, c * n:(c + 1) * n]
                        if b_flip == 0:
                            nc.vector.tensor_scalar_add(
                                out=dst, in0=pout[:],
                                scalar1=biast[:, t:t + 1])
                        else:
                            nc.scalar.add(out=dst, in_=pout[:],
                                          add=biast[:, t:t + 1])
                        b_flip ^= 1
                for a in range(4):
                    t = 4 * j + a
                    nc.sync.dma_start(out=out_d[128 * t:128 * (t + 1), :],
                                      in_=outts[a][:])
    nc.compile()
    return nc


def host_weights(w_upper, w_lower, bias, np_io):
    """Host-side weight layouts (lhsT conventions, see build())."""
    w_upper = np.asarray(w_upper, dtype=np.float32)
    w_lower = np.asarray(w_lower, dtype=np.float32)
    bias = np.asarray(bias, dtype=np.float32)
    wd = w_lower - w_upper
    wdT = wd.transpose(0, 2, 1)        # [d, i, o]
    wuT = w_upper.transpose(0, 2, 1)
    wdT4 = wdT.reshape(NT, 4, 32, 32)      # [t, a, i, o]

    # V[t]: lhsT[32a'+i, 32a+o] = wdT[4t+a][i,o] for a' <= a
    V5 = np.zeros((NT, 4, 32, 4, 32), np.float32)
    for a in range(4):
        for ap_ in range(a + 1):
            V5[:, ap_, :, a, :] = wdT4[:, a]
    V = np.ascontiguousarray(
        V5.reshape(NT, 128, 128).transpose(1, 0, 2).reshape(128, NT * 128))

    # WF: fused [wd_t; wu_t] block per tile; tile t at partition rows
    # 64*(t%2), column block t//2.  Block [i, 32a+o] = w_[4t+a][o, i].
    WF = np.zeros((128, (NT // 2) * 128), np.float32)
    for t in range(NT):
        bd = wdT[4 * t:4 * t + 4].transpose(1, 0, 2).reshape(32, 128)
        bu = wuT[4 * t:4 * t + 4].transpose(1, 0, 2).reshape(32, 128)
        row, col = 64 * (t % 2), 128 * (t // 2)
        WF[row:row + 32, col:col + 128] = bd
        WF[row + 32:row + 64, col:col + 128] = bu

    I32 = np.eye(32, dtype=np.float32)
    SUM4 = np.zeros((128, 4, 128), np.float32)
    for a in range(4):
        SUM4[:, a, 32 * a:32 * (a + 1)] = np.tile(I32, (4, 1))
    LT4 = np.kron(np.triu(np.ones((4, 4), np.float32)), I32)
    GW = np.stack([SUM4[:, a] @ LT4 for a in range(4)], axis=1)  # [128,4,128]
    O4I = np.vstack([np.zeros((96, 128), np.float32), np.tile(I32, (1, 4))])
    BIAST = np.ascontiguousarray(
        bias.reshape(NT, 4, 32).transpose(1, 2, 0).reshape(128, NT))
    return {
        "v": V.astype(np_io, copy=False),
        "wf": np.ascontiguousarray(WF).astype(np_io, copy=False),
        "gw": np.ascontiguousarray(GW.reshape(128, 512)).astype(
            np_io, copy=False),
        "o4i": O4I.astype(np_io, copy=False),
        "biast": BIAST,
    }


def _run_profiled(nc, in_maps):
    """Mirror of bass_utils' axon trace branch; the antenv.axon_hooks
    module is absent in this image, so drive the ctypes NTFF hook from
    trn_agent_boot directly and post-process with bass_utils helpers."""
    import glob
    import tempfile

    import gauge.profiler
    from concourse import bass2jax
    from concourse._compat import FishPath
    from trn_agent_boot.trn_boot import _ntff_profile_via_ctypes

    hook = _ntff_profile_via_ctypes("/opt/axon/libaxon_pjrt.so")
    if hook is None:
        raise RuntimeError("no NTFF profile symbols in libaxon_pjrt.so")
    neff_dir = tempfile.mkdtemp(prefix="dyad_prof_")
    with hook(neff_dir, [0]):
        results = bass2jax.run_bass_via_pjrt(nc, in_maps, n_cores=NCORES)
    ntffs = glob.glob(os.path.join(neff_dir, "*_body*.ntff"))
    if not ntffs:
        raise RuntimeError(f"no NTFFs in {neff_dir}")
    profile = gauge.profiler.Profile(
        profile_path=FishPath(neff_dir),
        kernel_dev_mode=True,
        profile_on_exit=False,
        bass_kernel=nc.m,
        offline_processing=True,
        fname="*_body*",
        metadata={},
    )
    return bass_utils._process_ntff_profile(
        profile, neff_dir, nc, list(range(NCORES)), [0], False, {},
        trace_events=False,
    ).as_bass_kernel_results(results)


def kernel(x, w_upper, w_lower, bias):
    global last_results
    (mm_dt,) = _cfg()
    key = (mm_dt,)
    if key not in _cache:
        _cache[key] = build(mm_dt)
    nc = _cache[key]

    np_io = mybir.dt.np(mm_dt)
    x = np.asarray(x, dtype=np.float32)
    w = host_weights(w_upper, w_lower, bias, np_io)
    in_maps = []
    for cidx in range(NCORES):
        xc = x[:, cidx * BC:(cidx + 1) * BC].astype(np_io)
        xs = np.ascontiguousarray(
            xc.reshape(NT, 128, BC).transpose(1, 0, 2).reshape(
                128, NT * BC))
        in_maps.append({"x": xs, **w})

    if os.environ.get("DYAD_TRACE", "0") == "1":
        try:
            res = _run_profiled(nc, in_maps)
        except Exception as e:  # profiling is best-effort
            print("profiled run failed (%s); falling back" % e)
            res = bass_utils.run_bass_kernel_spmd(
                nc, in_maps, core_ids=list(range(NCORES)), trace=False)
    else:
        res = bass_utils.run_bass_kernel_spmd(
            nc, in_maps, core_ids=list(range(NCORES)), trace=False)
    last_results = res
    out = np.concatenate([res.results[c]["out"] for c in range(NCORES)],
                         axis=1)
    return np.ascontiguousarray(out, dtype=np.float32)


# revision 4
# speedup vs baseline: 1.1029x; 1.1029x over previous
"""Trainium2 Bass kernel for the DyadBlock problem — column-pipelined.

Math (reference):
    xb   = x.reshape(DY, DI, B)
    incl = cumsum(xb, axis=0)
    out[d] = w_lower[d] @ incl[d] + w_upper[d] @ (total - incl[d]) + bias
           = wd[d] @ incl[d] + w_upper[d] @ total + bias      (wd = wl - wu)

Per tile t (4 dyads, 128 out rows), per batch-column chunk c (512 cols):
    out[t,c] = V[t].T @ x[t,c]            # local cumsum folded into V (K=128)
             + WF[t].T @ [C(t); T]        # carry AND total fused: one K=64
                                          # partial-array matmul
             + bias                       # fused into the PSUM drain
C(t) = sum of tile totals < t, T = grand total (per chunk).

Phase A chain runs at PAIR granularity so its SBUF layout directly
provides the [C;T] operand windows with zero data movement:
    state[j2] = [P(2j2); T-hole; P(2j2+1); T-hole]      (4 x 32 partitions)
      strips {0:32},{64:96} are placed by the chain weights (GW0/GW1/CR);
      hole strips drain as zeros and are later filled with T by
      log-doubling SBUF->SBUF DMAs (T = P(63) = state[31] strip {64:96}).
    window [0:64]  of block j2 = [C(2j2+1); T]  -> operand for odd tiles
    window [64:128] of block j2 = [C(2j2+2); T] -> operand for even tiles
    block 32 (zstate: zeros + T) serves tile 0.

PSUM legality (bisected in an earlier session): a partial-array matmul
must be the LAST write into its PSUM bank after full-array accumulation:
    phase A bank: GW0 (start) -> GW1 -> CR (K=32 partial @(64,0), stop)
    phase B bank: V (start, full) -> fused (K=64 partial @(row,0), stop)

Pipelining: x ships chunk-major; out(c0) DMA (sync queue) overlaps the
in(c1) stream (measured ~465 GB/s aggregate across the two queues vs
~350 for one).  Weights go first on the in queue (3.3MB) so phase B
never stalls on them.  fp16 operands/IO (PSUM fp32).
"""

import os

import numpy as np

import concourse.bacc as bacc
import concourse.mybir as mybir
import concourse.tile as tile
from concourse import bass_utils
from concourse.tile_rust import add_dep_helper

DY, DO, DI = 256, 32, 32
B = 8192
NCORES = 8
BC = B // NCORES   # batch columns per core
NT = DY // 4       # 64 row tiles (4 dyads each)
NP = NT // 2       # 32 tile pairs (phase A chain steps)

_cache = {}
last_results = None


def _cfg():
    mm = os.environ.get("DYAD_MM_DT", "fp16")
    mm_dt = {
        "f32": mybir.dt.float32,
        "bf16": mybir.dt.bfloat16,
        "fp16": mybir.dt.float16,
    }[mm]
    return (mm_dt,)


def _wfb(t):
    """wf column block and partition row for tile t."""
    if t == 0:
        return 0, 64
    if t % 2 == 1:
        return (t + 1) // 2, 0
    return t // 2, 64


def build(mm_dt, bc=BC, n=512):
    f32 = mybir.dt.float32
    nch = bc // n
    nc = bacc.Bacc("TRN2", target_bir_lowering=False, debug=False,
                   num_devices=NCORES)
    x_d = nc.dram_tensor("x", [128, nch * NT * n], mm_dt,
                         kind="ExternalInput").ap()
    v_d = nc.dram_tensor("v", [128, NT * 128], mm_dt,
                         kind="ExternalInput").ap()
    wf_d = nc.dram_tensor("wf", [128, 33 * 128], mm_dt,
                          kind="ExternalInput").ap()
    gw_d = nc.dram_tensor("gw", [128, 2 * 128], mm_dt,
                          kind="ExternalInput").ap()
    cr_d = nc.dram_tensor("cr", [128, 128], mm_dt, kind="ExternalInput").ap()
    bias_d = nc.dram_tensor("biast", [128, NT], f32, kind="ExternalInput").ap()
    # out is [p, t, col] in DRAM; host transposes to [t*128, col]
    out_d = nc.dram_tensor("out", [128, NT, bc], mm_dt,
                           kind="ExternalOutput").ap()

    with tile.TileContext(nc) as tc:
        with tc.tile_pool(name="wpool", bufs=1) as wp, \
             tc.tile_pool(name="xpool", bufs=12) as xp, \
             tc.tile_pool(name="opool", bufs=3) as op, \
             tc.tile_pool(name="psum", bufs=8, space="PSUM") as pp:
            gw = wp.tile([128, 2 * 128], mm_dt)
            cr = wp.tile([128, 128], mm_dt)
            biast = wp.tile([128, NT], f32)
            v = wp.tile([128, NT * 128], mm_dt)
            wf = wp.tile([128, 33 * 128], mm_dt)
            states = [wp.tile([128, 33 * n], mm_dt, name=f"state_{c}")
                      for c in range(nch)]

            nc.sync.dma_start(out=gw[:], in_=gw_d)
            nc.sync.dma_start(out=cr[:], in_=cr_d)
            nc.sync.dma_start(out=biast[:], in_=bias_d)

            # in-queue order: wf, V, then x chunk 0, then x chunk 1
            nc.gpsimd.dma_start(out=wf[:], in_=wf_d)
            QW = NT * 32
            for q in range(4):
                nc.gpsimd.dma_start(out=v[:, q * QW:(q + 1) * QW],
                                    in_=v_d[:, q * QW:(q + 1) * QW])
            octs = [[None] * 8 for _ in range(nch)]
            for c in range(nch):
                for o in range(8):
                    t0 = (c * NT + o * 8) * n
                    oc = xp.tile([128, 8 * n], mm_dt, tag="x",
                                 name=f"xoct_{c}_{o}")
                    nc.gpsimd.dma_start(out=oc[:], in_=x_d[:, t0:t0 + 8 * n])
                    octs[c][o] = oc

            def xsl(c, t):
                return octs[c][t // 8][:, (t % 8) * n:(t % 8) * n + n]

            # zstate zeros (block 32 strips {0:32} and {64:96})
            for c in range(nch):
                nc.vector.memset(states[c][0:32, 32 * n:33 * n], 0)
                nc.vector.memset(states[c][64:96, 32 * n:33 * n], 0)

            ab_flip = [0]

            def drain(dst, src, bias_ap=None):
                if ab_flip[0] == 0:
                    if bias_ap is None:
                        nc.vector.tensor_copy(out=dst, in_=src)
                    else:
                        nc.vector.tensor_scalar_add(out=dst, in0=src,
                                                    scalar1=bias_ap)
                else:
                    if bias_ap is None:
                        nc.scalar.copy(out=dst, in_=src)
                    else:
                        nc.scalar.add(out=dst, in_=src, add=bias_ap)
                ab_flip[0] ^= 1

            def phase_a_step(c, j2):
                sp = pp.tile([128, n], f32, tag="ps", name=f"sp_{c}_{j2}")
                m0 = nc.tensor.matmul(sp[:], gw[:, 0:128], xsl(c, 2 * j2),
                                      start=True, stop=False,
                                      tile_position=(0, 0))
                m1 = nc.tensor.matmul(sp[:], gw[:, 128:256],
                                      xsl(c, 2 * j2 + 1),
                                      start=False, stop=(j2 == 0),
                                      tile_position=(0, 0))
                add_dep_helper(m1.ins, m0.ins, sync=False,
                               reason="psum order gw0->gw1")
                if j2 > 0:
                    mc = nc.tensor.matmul(
                        sp[:], cr[64:96, :],
                        states[c][64:96, (j2 - 1) * n:j2 * n],
                        start=False, stop=True, tile_position=(64, 0),
                        skip_group_check=True)
                    add_dep_helper(mc.ins, m1.ins, sync=False,
                                   reason="psum order gw1->cr")
                drain(states[c][:, j2 * n:(j2 + 1) * n], sp[:])

            def t_broadcast(c):
                st = states[c]
                # T = P(63) at [64:96, 31n:32n]; log-double into [32:64, :]
                nc.gpsimd.dma_start(out=st[32:64, 0:n],
                                    in_=st[64:96, 31 * n:32 * n])
                w = 1
                while w < 32:
                    nc.gpsimd.dma_start(out=st[32:64, w * n:2 * w * n],
                                        in_=st[32:64, 0:w * n])
                    w *= 2
                nc.gpsimd.dma_start(out=st[32:64, 32 * n:33 * n],
                                    in_=st[32:64, 0:n])
                nc.gpsimd.dma_start(out=st[96:128, :], in_=st[32:64, :])

            oq = {}

            def phase_b_tile(c, t):
                pout = pp.tile([128, n], f32, tag="ps", name=f"po_{c}_{t}")
                mV = nc.tensor.matmul(pout[:], v[:, 128 * t:128 * (t + 1)],
                                      xsl(c, t), start=True, stop=False,
                                      tile_position=(0, 0))
                wb, row = _wfb(t)
                if t == 0:
                    opnd = states[c][64:128, 32 * n:33 * n]
                elif t % 2 == 1:
                    opnd = states[c][0:64, ((t - 1) // 2) * n:
                                     ((t - 1) // 2 + 1) * n]
                else:
                    opnd = states[c][64:128, (t // 2 - 1) * n:(t // 2) * n]
                mF = nc.tensor.matmul(
                    pout[:], wf[row:row + 64, 128 * wb:128 * wb + 128],
                    opnd, start=False, stop=True, tile_position=(row, 0),
                    skip_group_check=True)
                add_dep_helper(mF.ins, mV.ins, sync=False,
                               reason="psum order V->fused")
                q = t // 4
                if t % 4 == 0:
                    oq[(c, q)] = op.tile([128, 4 * n], mm_dt, tag="out",
                                         name=f"oq_{c}_{q}")
                drain(oq[(c, q)][:, (t % 4) * n:(t % 4 + 1) * n], pout[:],
                      bias_ap=biast[:, t:t + 1])
                if t % 4 == 3:
                    nc.sync.dma_start(
                        out=out_d[:, 4 * q:4 * q + 4, c * n:(c + 1) * n],
                        in_=oq[(c, q)][:])

            # ---- schedule ----
            for j2 in range(NP):
                phase_a_step(0, j2)
            t_broadcast(0)
            for j2 in range(NP):
                phase_a_step(1, j2)
                phase_b_tile(0, 2 * j2)
                phase_b_tile(0, 2 * j2 + 1)
            t_broadcast(1)
            for t in range(NT):
                phase_b_tile(1, t)
    nc.compile()
    return nc


def host_weights(w_upper, w_lower, bias, np_io):
    """Host-side weight layouts (lhsT conventions, see build())."""
    w_upper = np.asarray(w_upper, dtype=np.float32)
    w_lower = np.asarray(w_lower, dtype=np.float32)
    bias = np.asarray(bias, dtype=np.float32)
    wd = w_lower - w_upper
    wdT = wd.transpose(0, 2, 1)        # [d, i, o]
    wuT = w_upper.transpose(0, 2, 1)
    wdT4 = wdT.reshape(NT, 4, 32, 32)      # [t, a, i, o]

    # V[t]: lhsT[32a'+i, 32a+o] = wdT[4t+a][i,o] for a' <= a
    V5 = np.zeros((NT, 4, 32, 4, 32), np.float32)
    for a in range(4):
        for ap_ in range(a + 1):
            V5[:, ap_, :, a, :] = wdT4[:, a]
    V = np.ascontiguousarray(
        V5.reshape(NT, 128, 128).transpose(1, 0, 2).reshape(128, NT * 128))

    # WF: [wd_t; wu_t] 64x128 block per tile at (row, colblock) = _wfb(t)
    WF = np.zeros((128, 33 * 128), np.float32)
    for t in range(NT):
        bd = wdT[4 * t:4 * t + 4].transpose(1, 0, 2).reshape(32, 128)
        bu = wuT[4 * t:4 * t + 4].transpose(1, 0, 2).reshape(32, 128)
        wb, row = _wfb(t)
        WF[row:row + 32, 128 * wb:128 * (wb + 1)] = bd
        WF[row + 32:row + 64, 128 * wb:128 * (wb + 1)] = bu

    # GW0: tile total of even tile -> strips {0:32} and {64:96}
    # GW1: tile total of odd tile  -> strip {64:96}
    # CR : prev P (strip {64:96} operand) -> strips {0:32} and {64:96}
    GW = np.zeros((128, 2, 128), np.float32)
    for a in range(4):
        for i in range(32):
            GW[32 * a + i, 0, i] = 1.0
            GW[32 * a + i, 0, 64 + i] = 1.0
            GW[32 * a + i, 1, 64 + i] = 1.0
    CR = np.zeros((128, 128), np.float32)
    for i in range(32):
        CR[64 + i, i] = 1.0
        CR[64 + i, 64 + i] = 1.0
    BIAST = np.ascontiguousarray(
        bias.reshape(NT, 4, 32).transpose(1, 2, 0).reshape(128, NT))
    return {
        "v": V.astype(np_io, copy=False),
        "wf": np.ascontiguousarray(WF).astype(np_io, copy=False),
        "gw": np.ascontiguousarray(GW.reshape(128, 256)).astype(
            np_io, copy=False),
        "cr": CR.astype(np_io, copy=False),
        "biast": BIAST,
    }


def _run_profiled(nc, in_maps):
    """Mirror of bass_utils' axon trace branch; the antenv.axon_hooks
    module is absent in this image, so drive the ctypes NTFF hook from
    trn_agent_boot directly and post-process with bass_utils helpers."""
    import glob
    import tempfile

    import gauge.profiler
    from concourse import bass2jax
    from concourse._compat import FishPath
    from trn_agent_boot.trn_boot import _ntff_profile_via_ctypes

    hook = _ntff_profile_via_ctypes("/opt/axon/libaxon_pjrt.so")
    if hook is None:
        raise RuntimeError("no NTFF profile symbols in libaxon_pjrt.so")
    neff_dir = tempfile.mkdtemp(prefix="dyad_prof_")
    with hook(neff_dir, [0]):
        results = bass2jax.run_bass_via_pjrt(nc, in_maps, n_cores=NCORES)
    ntffs = glob.glob(os.path.join(neff_dir, "*_body*.ntff"))
    if not ntffs:
        raise RuntimeError(f"no NTFFs in {neff_dir}")
    profile = gauge.profiler.Profile(
        profile_path=FishPath(neff_dir),
        kernel_dev_mode=True,
        profile_on_exit=False,
        bass_kernel=nc.m,
        offline_processing=True,
        fname="*_body*",
        metadata={},
    )
    return bass_utils._process_ntff_profile(
        profile, neff_dir, nc, list(range(NCORES)), [0], False, {},
        trace_events=False,
    ).as_bass_kernel_results(results)


def kernel(x, w_upper, w_lower, bias):
    global last_results
    (mm_dt,) = _cfg()
    key = (mm_dt,)
    if key not in _cache:
        _cache[key] = build(mm_dt)
    nc = _cache[key]

    np_io = mybir.dt.np(mm_dt)
    n = 512
    nch = BC // n
    x = np.asarray(x, dtype=np.float32)
    w = host_weights(w_upper, w_lower, bias, np_io)
    in_maps = []
    for cidx in range(NCORES):
        xc = x[:, cidx * BC:(cidx + 1) * BC].astype(np_io)
        # [128, chunk, tile, n] chunk-major then tile-major
        xs = np.ascontiguousarray(
            xc.reshape(NT, 128, nch, n).transpose(1, 2, 0, 3).reshape(
                128, nch * NT * n))
        in_maps.append({"x": xs, **w})

    if os.environ.get("DYAD_TRACE", "0") == "1":
        try:
            res = _run_profiled(nc, in_maps)
        except Exception as e:  # profiling is best-effort
            print("profiled run failed (%s); falling back" % e)
            res = bass_utils.run_bass_kernel_spmd(
                nc, in_maps, core_ids=list(range(NCORES)), trace=False)
    else:
        res = bass_utils.run_bass_kernel_spmd(
            nc, in_maps, core_ids=list(range(NCORES)), trace=False)
    last_results = res
    outs = []
    for c in range(NCORES):
        o = res.results[c]["out"]  # [128, NT, bc]
        outs.append(np.asarray(o).transpose(1, 0, 2).reshape(NT * 128, BC))
    out = np.concatenate(outs, axis=1)
    return np.ascontiguousarray(out, dtype=np.float32)
